# revision 19
# baseline (speedup 1.0000x reference)
"""Trainium2 Bass kernel: per-(image, channel) class-mean replacement (segment mean + gather).

Input:  img [8, 128, 256, 256] f32, gt [8, 1, 256, 256] int32 (labels in [0, 21))
Output: out[b, c, h, w] = mean over pixels p of img[b, c, p] where gt[b, p] == gt[b, h, w]

Sharding: data-parallel over batch — each of the 8 NeuronCores processes one image.

Per-core algorithm (C=128 channels on partitions, HW=65536 pixels on free axis):
  Setup:    gt -> gtT [128pix, 512chunk] via PE transposes; class-major one-hot
            planes stash[p, c*512+col] = (gtT[p,col]==c), 21 wide DVE is_equal ops.
  Phase 1:  PE-transpose img chunks (f32); copy PSUM->SBUF with f32->bf16 cast;
            sums matmul SWAPPED: stationary = imgT chunk [128px,128ch], moving =
            one-hot view [128px,21cls] -> accumulate sumsT[128ch,21cls] in PSUM
            (tiny 21-col outputs). Counts via lhsT=onehot, rhs=ones -> cnt[21,1].
            Phase-2 one-hot transposes for the first PRE_G groups are
            interleaved here (they depend only on gt).
  Means:    sumsT -> SBUF -> PE-transpose -> meansT[21,128] bf16 = sums*rcp(cnt).
  Phase 2:  out[128ch,512px] = meansT^T @ ohT[21,512] per group; copy PSUM->SBUF
            as bf16; DMA out 2048-px tiles. Output DRAM tensor is bf16 (host
            casts back to f32) — halves write bandwidth at zero added error
            since means are already bf16. Pre-transposed groups run first so the
            out-DMA stream starts immediately after means; the remaining groups'
            transposes overlap the stream.
"""

import os
import sys

for _p in ("/opt/trn_rl_repo", "/root/.axon_site/_ro/trn_rl_repo"):
    if os.path.isdir(_p) and _p not in sys.path:
        sys.path.append(_p)

import numpy as np

P = 128          # channels == SBUF partitions
HW = 256 * 256   # pixels per image
NCLS = 21
CH = 128         # pixels per matmul chunk
NCH = HW // CH   # 512 chunks
FB = 2048        # pixels per DMA tile
NB = HW // FB    # 32 big tiles
NGR = HW // 512  # 128 phase-2 groups (512 px each)
PRE_G = 100      # groups whose ohT is pre-transposed during phase 1
EPS = 1e-8
N_CORES = 8

_CACHE = {}


def _build_module():
    import concourse.bacc as bacc
    import concourse.mybir as mybir
    import concourse.tile as tile
    from concourse.masks import make_identity

    f32 = mybir.dt.float32
    bf16 = mybir.dt.bfloat16
    i32 = mybir.dt.int32
    EQ = mybir.AluOpType.is_equal
    MULT = mybir.AluOpType.mult

    nc = bacc.Bacc("TRN2", target_bir_lowering=False, debug=False)
    img = nc.dram_tensor("img", [P, HW], f32, kind="ExternalInput")
    gt = nc.dram_tensor("gt", [HW], i32, kind="ExternalInput")
    out = nc.dram_tensor("out", [P, HW], bf16, kind="ExternalOutput")

    with tile.TileContext(nc) as tc:
        with (
            tc.tile_pool(name="constp", bufs=1) as constp,
            tc.tile_pool(name="imgp", bufs=5) as imgp,
            tc.tile_pool(name="rhsp", bufs=10) as rhsp,
            tc.tile_pool(name="ohsbp", bufs=3) as ohsbp,
            tc.tile_pool(name="outp", bufs=5) as outp,
            tc.tile_pool(name="psA", bufs=4, space="PSUM") as psA,
            tc.tile_pool(name="psB", bufs=1, space="PSUM") as psB,
            tc.tile_pool(name="psC", bufs=2, space="PSUM") as psC,
        ):
            # ---- constants ----
            ident32 = constp.tile([P, P], f32, tag="id32")
            make_identity(nc, ident32[:])
            ident16 = constp.tile([P, P], bf16, tag="id16")
            nc.vector.tensor_copy(out=ident16[:], in_=ident32[:])
            ones1 = constp.tile([P, 1], bf16, tag="ones1")
            nc.vector.memset(ones1[:], 1.0)

            # gt transposed to [128 pix, 512 chunk]: load gt naturally
            # [32, 2048], cast f32, PE-transpose 16 blocks [32,128]->[128,32].
            # gt staging lives in imgp slots (same per-partition footprint as an
            # img tile) so the big SBUF budget goes to ohstash instead.
            gtn_i = imgp.tile([32, HW // 32], i32, tag="img")
            nc.scalar.dma_start(
                out=gtn_i[:], in_=gt.ap().rearrange("(p f) -> p f", p=32)
            )
            gtn = imgp.tile([32, HW // 32], f32, tag="img")
            nc.scalar.copy(out=gtn[:], in_=gtn_i[:])
            # gtT in CHUNK order: gtT[:, gc] = labels of chunk gc. Transpose
            # block b yields chunks {16r+b}, scattered via a stride-16 dest AP.
            gtT = constp.tile([P, NCH], f32, tag="gtT")
            gtTv = gtT[:].rearrange("p (r b) -> p r b", b=16)
            for b in range(16):
                gps = psC.tile([P, 32], f32, tag="c")
                nc.tensor.transpose(
                    out=gps[:],
                    in_=gtn[:, b * P : (b + 1) * P],
                    identity=ident32[0:32, 0:32],
                )
                nc.vector.tensor_copy(out=gtTv[:, :, b], in_=gps[:])

            # class-major one-hot planes: stash[p, c*NCH + gc] = (gtT[p,gc]==c),
            # built in 64-chunk column spans (span s gates only tiles 4s..4s+3,
            # so the first sums matmul can start early). Spans are issued
            # just-in-time from the tile loop so they don't clog the in-order
            # DVE/Pool queues ahead of the copy pipeline.
            stash = constp.tile([P, NCLS * NCH], bf16, tag="stash")

            def issue_span(s):
                eng = nc.vector if s % 2 == 0 else nc.gpsimd
                for c in range(NCLS):
                    eng.tensor_scalar(
                        stash[:, c * NCH + 64 * s : c * NCH + 64 * (s + 1)],
                        gtT[:, 64 * s : 64 * (s + 1)],
                        float(c),
                        None,
                        EQ,
                    )

            issue_span(0)
            issue_span(1)
            stashv = stash[:].rearrange("p (c j) -> p c j", c=NCLS)

            def ohview(gc):
                return stashv[:, :, gc]  # [128px, 21cls]

            # pre-transposed ohT storage for groups [0, PRE_G)
            ohstash = constp.tile([32, PRE_G * 512], bf16, tag="ohstash")

            sums = psB.tile([P, NCLS], f32, tag="sums")
            cntt = psB.tile([NCLS, 1], f32, tag="cnt")
            cnt = cntt[:]

            def copy_by(eng, dst, src):
                if eng == 0:
                    nc.vector.tensor_copy(out=dst, in_=src)
                elif eng == 1:
                    nc.scalar.copy(out=dst, in_=src)
                else:
                    nc.gpsimd.tensor_copy(out=dst, in_=src)

            def pre_transpose_group(g, eng):
                ohps = psC.tile([32, 512], bf16, tag="c")
                for q in range(4):
                    nc.tensor.transpose(
                        out=ohps[0:NCLS, q * CH : (q + 1) * CH],
                        in_=ohview(g * 4 + q),
                        identity=ident16[:],
                    )
                copy_by(eng, ohstash[0:NCLS, g * 512 : (g + 1) * 512], ohps[0:NCLS, :])

            # ---- phase 1: per-class sums + counts (swapped matmuls) ----
            # Software-pipelined: the sums matmuls for 512-px group g are
            # issued on the PE queue two groups late, so PE never blocks
            # in-order on the PSUM->SBUF copy of the group it just transposed.
            LAG = 2
            pending = []  # (g4, rhs4 tile) awaiting their sums matmuls

            def issue_sums(g4, rhs4):
                for q in range(4):
                    gc = g4 * 4 + q
                    nc.tensor.matmul(
                        out=sums[:],
                        lhsT=rhs4[:, q * CH : (q + 1) * CH],
                        rhs=ohview(gc),
                        start=(gc == 0),
                        stop=(gc == NCH - 1),
                    )
                    nc.tensor.matmul(
                        out=cnt,
                        lhsT=ohview(gc),
                        rhs=ones1[:],
                        start=(gc == 0),
                        stop=(gc == NCH - 1),
                    )

            pre_done = 0
            for t in range(NB):
                # one-hot span for tiles [4(s), 4(s)+3] issued 2 spans ahead
                if t % 4 == 0 and t // 4 + 2 < 8:
                    issue_span(t // 4 + 2)
                ib = imgp.tile([P, FB], f32, tag="img")
                for h in range(2):
                    nc.sync.dma_start(
                        out=ib[:, h * 1024 : (h + 1) * 1024],
                        in_=img.ap()[:, t * FB + h * 1024 : t * FB + (h + 1) * 1024],
                    )
                for jj in range(4):
                    g4 = t * 4 + jj
                    tp4 = psA.tile([P, 512], f32, tag="a")
                    for q in range(4):
                        nc.tensor.transpose(
                            out=tp4[:, q * CH : (q + 1) * CH],
                            in_=ib[:, (jj * 4 + q) * CH : (jj * 4 + q + 1) * CH],
                            identity=ident32[:],
                        )
                    rhs4 = rhsp.tile([P, 512], bf16, tag="rhs")
                    copy_by((0, 1, 2)[g4 % 3], rhs4[:], tp4[:])
                    pending.append((g4, rhs4))
                    if len(pending) > LAG:
                        issue_sums(*pending.pop(0))
                # interleave phase-2 ohT pre-transposes (depend only on gt)
                target = min(PRE_G, ((t + 1) * PRE_G) // NB)
                while pre_done < target:
                    pre_transpose_group(pre_done, (1, 0, 1)[pre_done % 3])
                    pre_done += 1
            while pending:
                issue_sums(*pending.pop(0))

            # ---- means: meansT[21,128] bf16 = sumsT^T * 1/(cnt+eps) ----
            sms = constp.tile([P, NCLS], f32, tag="sms")
            nc.vector.tensor_copy(out=sms[:], in_=sums[:])
            smsP = psC.tile([NCLS, P], f32, tag="c")
            nc.tensor.transpose(out=smsP[:], in_=sms[:], identity=ident32[:])
            cnte = constp.tile([NCLS, 1], f32, tag="cnte")
            nc.vector.tensor_scalar_add(cnte[:], cnt, EPS)
            rcp = constp.tile([NCLS, 1], f32, tag="rcp")
            nc.vector.reciprocal(out=rcp[:], in_=cnte[:])
            meansT = constp.tile([NCLS, P], bf16, tag="meansT")
            nc.vector.tensor_scalar(meansT[:], smsP[:], rcp[:, 0:1], None, MULT)

            # ---- phase 2: out[128ch, px] = meansT^T @ ohT ----
            # JIT output tiles (no pre-transposed ohT) are spread between
            # pre-transposed tiles so their extra transpose+copy load evens out
            # over the whole out-stream.
            n_pre_t = PRE_G // 4
            tile_order, pi, ji = [], 0, n_pre_t
            for k in range(NB):
                if k % 4 == 3 and ji < NB:
                    tile_order.append(ji)
                    ji += 1
                else:
                    tile_order.append(pi)
                    pi += 1
            cp_i = 0  # global 3-engine rotation for all phase-2 copies
            for tt in tile_order:
                jit = tt >= n_pre_t
                if jit:
                    ohs_pair = []
                    for half in range(2):
                        # one [32,1024] PSUM tile = ohT for a PAIR of groups
                        ohps2 = psC.tile([32, 1024], bf16, tag="c")
                        for qq in range(8):
                            nc.tensor.transpose(
                                out=ohps2[0:NCLS, qq * CH : (qq + 1) * CH],
                                in_=ohview((4 * tt + 2 * half) * 4 + qq),
                                identity=ident16[:],
                            )
                        ohs = ohsbp.tile([32, 1024], bf16, tag="oh")
                        copy_by((0, 1, 2)[cp_i % 3], ohs[0:NCLS, :], ohps2[0:NCLS, :])
                        cp_i += 1
                        ohs_pair.append(ohs)
                ob4 = outp.tile([P, FB], bf16, tag="ob")
                for k in range(4):
                    g = 4 * tt + k
                    if jit:
                        rhs_ap = ohs_pair[k // 2][0:NCLS, (k % 2) * 512 : (k % 2 + 1) * 512]
                    else:
                        rhs_ap = ohstash[0:NCLS, g * 512 : (g + 1) * 512]
                    op_ = psA.tile([P, 512], f32, tag="a")
                    nc.tensor.matmul(
                        out=op_[:], lhsT=meansT[:], rhs=rhs_ap, start=True, stop=True
                    )
                    copy_by((0, 1, 2)[cp_i % 3], ob4[:, k * 512 : (k + 1) * 512], op_[:])
                    cp_i += 1
                if tt == tile_order[-1]:
                    # split the last tile's DMA so the tail drains sooner
                    for s in range(4):
                        nc.sync.dma_start(
                            out=out.ap()[:, (4 * tt + s) * 512 : (4 * tt + s + 1) * 512],
                            in_=ob4[:, s * 512 : (s + 1) * 512],
                        )
                else:
                    nc.sync.dma_start(
                        out=out.ap()[:, tt * FB : (tt + 1) * FB], in_=ob4[:]
                    )

    nc.compile()
    return nc


def get_module():
    if "nc" not in _CACHE:
        _CACHE["nc"] = _build_module()
    return _CACHE["nc"]


def kernel(img, gt):
    from concourse.bass_utils import run_bass_kernel_spmd

    img = np.asarray(img)
    gt = np.asarray(gt)
    B, C, H, W = img.shape
    assert (B, C, H * W) == (N_CORES, P, HW), (img.shape,)
    img2 = np.ascontiguousarray(img.reshape(B, C, H * W))
    gt2 = np.ascontiguousarray(gt.reshape(B, H * W))

    nc = get_module()
    in_maps = [{"img": img2[i], "gt": gt2[i]} for i in range(B)]
    res = run_bass_kernel_spmd(nc, in_maps, core_ids=list(range(N_CORES)))
    out = np.stack(
        [np.asarray(res.results[i]["out"]).astype(np.float32) for i in range(B)],
        axis=0,
    )
    return out.reshape(B, C, H, W)


if __name__ == "__main__":
    rng = np.random.default_rng(0)
    img = rng.standard_normal((8, 128, 256, 256), dtype=np.float32)
    gt = rng.integers(0, NCLS, size=(8, 1, 256, 256), dtype=np.int32)
    out = kernel(img=img, gt=gt)
    print("out", out.shape, out.dtype)


# revision 24
# speedup vs baseline: 1.0677x; 1.0677x over previous
"""Trainium2 Bass kernel: per-(image, channel) class-mean replacement (segment mean + gather).

Input:  img [8, 128, 256, 256] f32, gt [8, 1, 256, 256] int32 (labels in [0, 21))
Output: out[b, c, h, w] = mean over pixels p of img[b, c, p] where gt[b, p] == gt[b, h, w]

Sharding: data-parallel over batch — each of the 8 NeuronCores processes one image.

Per-core algorithm (C=128 channels on partitions, HW=65536 pixels on free axis):
  Setup:    gt -> gtT [128pix, 512chunk] via PE transposes; class-major one-hot
            planes stash[p, c*512+col] = (gtT[p,col]==c), 21 wide DVE is_equal ops.
  Phase 1:  PE-transpose img chunks (f32); copy PSUM->SBUF with f32->bf16 cast;
            sums matmul SWAPPED: stationary = imgT chunk [128px,128ch], moving =
            one-hot view [128px,21cls] -> accumulate sumsT[128ch,21cls] in PSUM
            (tiny 21-col outputs). Counts via lhsT=onehot, rhs=ones -> cnt[21,1].
            Phase-2 one-hot transposes for the first PRE_G groups are
            interleaved here (they depend only on gt).
  Means:    sumsT -> SBUF -> PE-transpose -> meansT[21,128] bf16 = sums*rcp(cnt).
  Phase 2:  out[128ch,512px] = meansT^T @ ohT[21,512] per group; copy PSUM->SBUF
            as bf16; DMA out 2048-px tiles. Output DRAM tensor is bf16 (host
            casts back to f32) — halves write bandwidth at zero added error
            since means are already bf16. Pre-transposed groups run first so the
            out-DMA stream starts immediately after means; the remaining groups'
            transposes overlap the stream.
"""

import os
import sys

for _p in ("/opt/trn_rl_repo", "/root/.axon_site/_ro/trn_rl_repo"):
    if os.path.isdir(_p) and _p not in sys.path:
        sys.path.append(_p)

import numpy as np

P = 128          # channels == SBUF partitions
HW = 256 * 256   # pixels per image
NCLS = 21
CH = 128         # pixels per matmul chunk
NCH = HW // CH   # 512 chunks
FB = 2048        # pixels per DMA tile
NB = HW // FB    # 32 big tiles
NGR = HW // 512  # 128 phase-2 groups (512 px each)
PRE_G = 100      # groups whose ohT is pre-transposed during phase 1
EPS = 1e-8
N_CORES = 8

_CACHE = {}


def _build_module():
    import concourse.bacc as bacc
    import concourse.mybir as mybir
    import concourse.tile as tile
    from concourse.masks import make_identity

    f32 = mybir.dt.float32
    bf16 = mybir.dt.bfloat16
    i32 = mybir.dt.int32
    EQ = mybir.AluOpType.is_equal
    MULT = mybir.AluOpType.mult

    nc = bacc.Bacc("TRN2", target_bir_lowering=False, debug=False)
    img = nc.dram_tensor("img", [P, HW], f32, kind="ExternalInput")
    gt = nc.dram_tensor("gt", [HW], i32, kind="ExternalInput")
    out = nc.dram_tensor("out", [P, HW], bf16, kind="ExternalOutput")

    with tile.TileContext(nc) as tc:
        with (
            tc.tile_pool(name="constp", bufs=1) as constp,
            tc.tile_pool(name="imgp", bufs=5) as imgp,
            tc.tile_pool(name="rhsp", bufs=10) as rhsp,
            tc.tile_pool(name="ohsbp", bufs=3) as ohsbp,
            tc.tile_pool(name="outp", bufs=5) as outp,
            tc.tile_pool(name="psA", bufs=4, space="PSUM") as psA,
            tc.tile_pool(name="psB", bufs=1, space="PSUM") as psB,
            tc.tile_pool(name="psC", bufs=2, space="PSUM") as psC,
        ):
            # ---- constants ----
            ident32 = constp.tile([P, P], f32, tag="id32")
            make_identity(nc, ident32[:])
            ident16 = constp.tile([P, P], bf16, tag="id16")
            nc.vector.tensor_copy(out=ident16[:], in_=ident32[:])
            ones1 = constp.tile([P, 1], bf16, tag="ones1")
            nc.vector.memset(ones1[:], 1.0)

            # gt transposed to [128 pix, 512 chunk]: load gt naturally
            # [32, 2048], cast f32, PE-transpose 16 blocks [32,128]->[128,32].
            # gt staging lives in imgp slots (same per-partition footprint as an
            # img tile) so the big SBUF budget goes to ohstash instead.
            gtn_i = imgp.tile([32, HW // 32], i32, tag="img")
            nc.scalar.dma_start(
                out=gtn_i[:], in_=gt.ap().rearrange("(p f) -> p f", p=32)
            )
            gtn = imgp.tile([32, HW // 32], f32, tag="img")
            nc.scalar.copy(out=gtn[:], in_=gtn_i[:])
            # gtT in CHUNK order: gtT[:, gc] = labels of chunk gc. Transpose
            # block b yields chunks {16r+b}, scattered via a stride-16 dest AP.
            gtT = constp.tile([P, NCH], f32, tag="gtT")
            gtTv = gtT[:].rearrange("p (r b) -> p r b", b=16)
            for b in range(16):
                gps = psC.tile([P, 32], f32, tag="c")
                nc.tensor.transpose(
                    out=gps[:],
                    in_=gtn[:, b * P : (b + 1) * P],
                    identity=ident32[0:32, 0:32],
                )
                nc.vector.tensor_copy(out=gtTv[:, :, b], in_=gps[:])

            # class-major one-hot planes: stash[p, c*NCH + gc] = (gtT[p,gc]==c),
            # built in 64-chunk column spans (span s gates only tiles 4s..4s+3,
            # so the first sums matmul can start early). Spans are issued
            # just-in-time from the tile loop so they don't clog the in-order
            # DVE/Pool queues ahead of the copy pipeline.
            stash = constp.tile([P, NCLS * NCH], bf16, tag="stash")

            def issue_span(s, eng):
                for c in range(NCLS):
                    eng.tensor_scalar(
                        stash[:, c * NCH + 64 * s : c * NCH + 64 * (s + 1)],
                        gtT[:, 64 * s : 64 * (s + 1)],
                        float(c),
                        None,
                        EQ,
                    )

            # span 0 on DVE (fast, gates the first sums matmul); the rest on
            # Pool, which is otherwise idle in phase 1 — span s gates only
            # tiles 4s.., all ready well before needed.
            issue_span(0, nc.vector)
            for s in range(1, 8):
                issue_span(s, nc.gpsimd)
            stashv = stash[:].rearrange("p (c j) -> p c j", c=NCLS)

            def ohview(gc):
                return stashv[:, :, gc]  # [128px, 21cls]

            # pre-transposed ohT storage for groups [0, PRE_G)
            ohstash = constp.tile([32, PRE_G * 512], bf16, tag="ohstash")

            sums = psB.tile([P, NCLS], f32, tag="sums")
            cntt = psB.tile([NCLS, 1], f32, tag="cnt")
            cnt = cntt[:]

            def copy_by(eng, dst, src):
                if eng == 0:
                    nc.vector.tensor_copy(out=dst, in_=src)
                elif eng == 1:
                    nc.scalar.copy(out=dst, in_=src)
                else:
                    nc.gpsimd.tensor_copy(out=dst, in_=src)

            def pre_transpose_group(g, eng):
                ohps = psC.tile([32, 512], bf16, tag="c")
                for q in range(4):
                    nc.tensor.transpose(
                        out=ohps[0:NCLS, q * CH : (q + 1) * CH],
                        in_=ohview(g * 4 + q),
                        identity=ident16[:],
                    )
                copy_by(eng, ohstash[0:NCLS, g * 512 : (g + 1) * 512], ohps[0:NCLS, :])

            # ---- phase 1: per-class sums + counts (swapped matmuls) ----
            # Software-pipelined: the sums matmuls for 512-px group g are
            # issued on the PE queue two groups late, so PE never blocks
            # in-order on the PSUM->SBUF copy of the group it just transposed.
            LAG = 2
            pending = []  # (g4, rhs4 tile) awaiting their sums matmuls

            def issue_sums(g4, rhs4):
                for q in range(4):
                    gc = g4 * 4 + q
                    nc.tensor.matmul(
                        out=sums[:],
                        lhsT=rhs4[:, q * CH : (q + 1) * CH],
                        rhs=ohview(gc),
                        start=(gc == 0),
                        stop=(gc == NCH - 1),
                    )
                    nc.tensor.matmul(
                        out=cnt,
                        lhsT=ohview(gc),
                        rhs=ones1[:],
                        start=(gc == 0),
                        stop=(gc == NCH - 1),
                    )

            pre_done = 0
            for t in range(NB):
                ib = imgp.tile([P, FB], f32, tag="img")
                for h in range(2):
                    nc.sync.dma_start(
                        out=ib[:, h * 1024 : (h + 1) * 1024],
                        in_=img.ap()[:, t * FB + h * 1024 : t * FB + (h + 1) * 1024],
                    )
                for jj in range(4):
                    g4 = t * 4 + jj
                    tp4 = psA.tile([P, 512], f32, tag="a")
                    for q in range(4):
                        nc.tensor.transpose(
                            out=tp4[:, q * CH : (q + 1) * CH],
                            in_=ib[:, (jj * 4 + q) * CH : (jj * 4 + q + 1) * CH],
                            identity=ident32[:],
                        )
                    rhs4 = rhsp.tile([P, 512], bf16, tag="rhs")
                    copy_by(g4 % 2, rhs4[:], tp4[:])
                    pending.append((g4, rhs4))
                    if len(pending) > LAG:
                        issue_sums(*pending.pop(0))
                # interleave phase-2 ohT pre-transposes (depend only on gt)
                target = min(PRE_G, ((t + 1) * PRE_G) // NB)
                while pre_done < target:
                    pre_transpose_group(pre_done, (0, 1, 0)[pre_done % 3])
                    pre_done += 1
            while pending:
                issue_sums(*pending.pop(0))

            # ---- means: meansT[21,128] bf16 = sumsT^T * 1/(cnt+eps) ----
            sms = constp.tile([P, NCLS], f32, tag="sms")
            nc.vector.tensor_copy(out=sms[:], in_=sums[:])
            smsP = psC.tile([NCLS, P], f32, tag="c")
            nc.tensor.transpose(out=smsP[:], in_=sms[:], identity=ident32[:])
            cnte = constp.tile([NCLS, 1], f32, tag="cnte")
            nc.vector.tensor_scalar_add(cnte[:], cnt, EPS)
            rcp = constp.tile([NCLS, 1], f32, tag="rcp")
            nc.vector.reciprocal(out=rcp[:], in_=cnte[:])
            meansT = constp.tile([NCLS, P], bf16, tag="meansT")
            nc.vector.tensor_scalar(meansT[:], smsP[:], rcp[:, 0:1], None, MULT)

            # ---- phase 2: out[128ch, px] = meansT^T @ ohT ----
            # JIT output tiles (no pre-transposed ohT) are spread between
            # pre-transposed tiles so their extra transpose+copy load evens out
            # over the whole out-stream.
            n_pre_t = PRE_G // 4
            tile_order, pi, ji = [], 0, n_pre_t
            for k in range(NB):
                if k % 4 == 3 and ji < NB:
                    tile_order.append(ji)
                    ji += 1
                else:
                    tile_order.append(pi)
                    pi += 1
            cp_i = 0  # global 3-engine rotation for all phase-2 copies
            for tt in tile_order:
                jit = tt >= n_pre_t
                if jit:
                    ohs_pair = []
                    for half in range(2):
                        # one [32,1024] PSUM tile = ohT for a PAIR of groups
                        ohps2 = psC.tile([32, 1024], bf16, tag="c")
                        for qq in range(8):
                            nc.tensor.transpose(
                                out=ohps2[0:NCLS, qq * CH : (qq + 1) * CH],
                                in_=ohview((4 * tt + 2 * half) * 4 + qq),
                                identity=ident16[:],
                            )
                        ohs = ohsbp.tile([32, 1024], bf16, tag="oh")
                        copy_by(0, ohs[0:NCLS, :], ohps2[0:NCLS, :])
                        ohs_pair.append(ohs)
                ob4 = outp.tile([P, FB], bf16, tag="ob")
                for k in range(4):
                    g = 4 * tt + k
                    if jit:
                        rhs_ap = ohs_pair[k // 2][0:NCLS, (k % 2) * 512 : (k % 2 + 1) * 512]
                    else:
                        rhs_ap = ohstash[0:NCLS, g * 512 : (g + 1) * 512]
                    op_ = psA.tile([P, 512], f32, tag="a")
                    nc.tensor.matmul(
                        out=op_[:], lhsT=meansT[:], rhs=rhs_ap, start=True, stop=True
                    )
                    eng = ((1, 1, 2, 2) if jit else (1, 0, 2, 1))[k]
                    copy_by(eng, ob4[:, k * 512 : (k + 1) * 512], op_[:])
                if tt == tile_order[-1]:
                    # split the last tile's DMA so the tail drains sooner
                    for s in range(4):
                        nc.sync.dma_start(
                            out=out.ap()[:, (4 * tt + s) * 512 : (4 * tt + s + 1) * 512],
                            in_=ob4[:, s * 512 : (s + 1) * 512],
                        )
                else:
                    nc.sync.dma_start(
                        out=out.ap()[:, tt * FB : (tt + 1) * FB], in_=ob4[:]
                    )

    nc.compile()
    return nc


def get_module():
    if "nc" not in _CACHE:
        _CACHE["nc"] = _build_module()
    return _CACHE["nc"]


def kernel(img, gt):
    from concourse.bass_utils import run_bass_kernel_spmd

    img = np.asarray(img)
    gt = np.asarray(gt)
    B, C, H, W = img.shape
    assert (B, C, H * W) == (N_CORES, P, HW), (img.shape,)
    img2 = np.ascontiguousarray(img.reshape(B, C, H * W))
    gt2 = np.ascontiguousarray(gt.reshape(B, H * W))

    nc = get_module()
    in_maps = [{"img": img2[i], "gt": gt2[i]} for i in range(B)]
    res = run_bass_kernel_spmd(nc, in_maps, core_ids=list(range(N_CORES)))
    out = np.stack(
        [np.asarray(res.results[i]["out"]).astype(np.float32) for i in range(B)],
        axis=0,
    )
    return out.reshape(B, C, H, W)


if __name__ == "__main__":
    rng = np.random.default_rng(0)
    img = rng.standard_normal((8, 128, 256, 256), dtype=np.float32)
    gt = rng.integers(0, NCLS, size=(8, 1, 256, 256), dtype=np.int32)
    out = kernel(img=img, gt=gt)
    print("out", out.shape, out.dtype)
